# revision 1
# baseline (speedup 1.0000x reference)
"""DualAttention (CAM + PAM) Trainium2 Bass kernel.

Problem (per batch b of 4, C=64 channels, N=8192 positions):
  CAM: A = x@x^T (64x64 gram); att_c = softmax(rowmax(A)-A, axis=0);
       cam = gamma_cam * att_c @ x + x
  PAM: q,k (8,N), v (64,N) via 1x1 convs; att_p = softmax(q^T k, axis=-1)
       pam = gamma_pam * v @ att_p^T + x
  out = cam + pam
Sharding: 8 cores = (batch b in 0..3) x (query-half h in 0..1); each core
streams the full 8192-wide key/value range flash-attention style.

I/O format (per-call tunnel traffic is part of the measured wall time, so
inputs are packed tight):
- x ships int16-quantized (xq = round(x/s), s = max|x|/32766, ones row =
  round(1/s)); quantization costs ~2e-4 rel err. The device reconstructs
  real-valued x = s * float(xq) on DVE (s ships replicated in a misc
  column), so everything downstream is numerically identical to the f32
  kernel. NOTE: folding s into the weights instead (so the device works on
  raw ints) looked algebraically equivalent and passed a float64 host
  emulation at 1e-4, but on hardware it produced a deterministic 1.9e-2
  error independent of which engine did the int conversion -- root cause
  never found (no profiler here). Keep the reconstruct-x form.
- the small constants ship consolidated in one misc [65,328] f32 tensor
  (aux | eye65 | wv | wq|wk | s), converted/sliced on device; 2 input
  tensors total instead of 6.
- y returns fp16 (~5e-4 rounding, gate is 2e-2).

Pipeline (unchanged from the f32 version): scores computed transposed so
the exp'd tile feeds the PV matmul as the moving operand; the softmax
denominator comes from a ones-column in the v-projection (PSUM row 64 of
the PV accumulator); fp32r for big matmuls; the exp stream on the scalar
engine is the bottleneck (~92% of span) with score fills one group ahead
and q/k/v projections, x^T transposes, the 64x64 gram and the CAM softmax
woven into PE idle cycles of the first six query tiles.
"""

import numpy as np

B, C, N = 4, 64, 8192
CQK = C // 8
NCORES = 8

_prog_cache = {}

# misc tensor column layout (all v1-scale values; x is reconstructed on
# device as s * float(xq), so no scale folding anywhere downstream)
_AUX = 0        # [0:64, 0:66]: 2*eye | gamma_cam (col 64) | gamma_pam (col 65)
_EYE = 66       # [65, 66:131]: eye(65)  (both transposes)
_WV = 131       # [65, 131:197]: wv (w3^T | b3 row | ones col), f32
_WQK = 197      # [65, 197:327]: wq | wk (w^T with bias rows), f32
_SCL = 327      # [65, 327:328]: s replicated down all partitions
_MISCW = 328


def _pcopy(nc, opts, out, in_):
    if opts.get("qk_copy_dve", True):
        nc.vector.tensor_copy(out, in_)
    else:
        nc.scalar.copy(out, in_)


def _build(Ntot, NH, opts=()):
    opts = dict(opts)
    import concourse.bass as bass  # noqa: F401
    import concourse.bacc as bacc
    import concourse.tile as tile
    from concourse import mybir
    from contextlib import ExitStack

    f32 = mybir.dt.float32
    f32r = mybir.dt.float32r
    f16 = mybir.dt.float16
    i16 = mybir.dt.int16
    bf16 = mybir.dt.bfloat16
    AF = mybir.ActivationFunctionType
    Alu = mybir.AluOpType
    X = mybir.AxisListType.X

    NCH = Ntot // 128      # 128-wide key chunks
    NT = NH // 512         # query tiles
    KT = Ntot // 512       # 512-wide column tiles of full range
    NHG = NCH // 2         # half-groups (2 chunks) per query tile  # noqa: F841

    interleave = opts.get("interleave", True)
    GS = opts.get("group_size", 2)
    st_bufs = opts.get("st_bufs", 3)
    pv_bufs = opts.get("pv_bufs", 1)
    misc_bufs = opts.get("misc_bufs", 1)
    GPT = (NCH + GS - 1) // GS  # groups per tile

    nc = bacc.Bacc("TRN2", target_bir_lowering=False, debug=False)
    xq_d = nc.dram_tensor("xq", [65, Ntot], i16, kind="ExternalInput")
    misc_d = nc.dram_tensor("misc", [65, _MISCW], f32, kind="ExternalInput")
    y_d = nc.dram_tensor("y", [64, NH], f16, kind="ExternalOutput")

    with tile.TileContext(nc) as tc, ExitStack() as ctx:
        sb = ctx.enter_context(tc.tile_pool(name="sb", bufs=1))
        ps = ctx.enter_context(tc.tile_pool(name="ps", bufs=1, space="PSUM"))
        pps = ctx.enter_context(tc.tile_pool(name="pps", bufs=2))
        tl = ctx.enter_context(tc.tile_pool(name="tl", bufs=2))

        # reconstructed real-valued x (= s * quantized ints), plain f32
        xr_sb = sb.tile([65, Ntot], f32)
        misc_sb = sb.tile([65, _MISCW], f32)
        wv_sb = sb.tile([65, 66], bf16)
        q_sb = sb.tile([CQK, NH], f32r)
        k_sb = sb.tile([CQK, Ntot], f32r)
        vT_sb = sb.tile([128, NCH, 65], f32r)
        xT_sb = sb.tile([128, NCH, 65], f32)
        cam_sb = sb.tile([64, NH], f32)
        # bf16 copy of xq for the v-projection (bf16 matmul streams 66 cols
        # at 1 cyc/row): converted on the idle Pool engine.
        xbf_sb = sb.tile([65, Ntot], bf16)
        # tiny dummy exp: triggers the one-time ACT table load immediately,
        # overlapping it with the input DMAs instead of the first real exp
        warm_sb = sb.tile([1, 2], f32)
        nc.vector.memset(warm_sb[:, :], 0.25)
        nc.scalar.activation(warm_sb[:, :], warm_sb[:, :], AF.Exp)

        nc.gpsimd.dma_start(misc_sb[:, :], misc_d[:, :])
        nc.vector.tensor_copy(wv_sb[:, :], misc_sb[:, _WV:_WV + 66])
        # int16 x streams through a small double-buffered staging tile;
        # each chunk is converted to f32 ints on DVE, rescaled by s into
        # real-valued x, then converted to bf16 on the idle Pool engine
        # for the v-projection.  Downstream is identical to the f32 kernel.
        XCH = 512
        for ci in range(Ntot // XCH):
            lo, hi = ci * XCH, (ci + 1) * XCH
            xst = sb.tile([65, XCH], i16, tag="xst", bufs=2, name="xst")
            nc.sync.dma_start(xst[:, :], xq_d[:, lo:hi])
            xtmp = sb.tile([65, XCH], f32, tag="xtmp", bufs=2, name="xtmp")
            nc.vector.tensor_copy(xtmp[:, :], xst[:, :])
            nc.vector.tensor_scalar(
                xr_sb[:, lo:hi], xtmp[:, :], misc_sb[:, _SCL:_SCL + 1],
                None, op0=Alu.mult,
            )
            nc.gpsimd.tensor_copy(xbf_sb[:, lo:hi], xr_sb[:, lo:hi])

        # ---- stage-1 emitters (each emits one batch when called) ----
        def em_qprod(t, first=False):
            # at startup the pv bank is still idle: borrowing it for the very
            # first q-projection (and copying via the idle ACT engine) breaks
            # the misc-slot serialization on the critical chain to exp(0)
            tag = "pv" if first else "misc"
            bufs = pv_bufs if first else misc_bufs
            qp = ps.tile([CQK, 512], f32, tag=tag, bufs=bufs, name="qp")
            nc.tensor.matmul(qp[:, :], misc_sb[:, _WQK:_WQK + CQK],
                             xr_sb[:, t * 512:(t + 1) * 512])
            if first:
                nc.scalar.copy(q_sb[:, t * 512:(t + 1) * 512], qp[:, :])
            else:
                _pcopy(nc, opts, q_sb[:, t * 512:(t + 1) * 512], qp[:, :])

        def em_kprod(g):
            kp = ps.tile([CQK, 512], f32, tag="misc", bufs=misc_bufs, name="kp")
            nc.tensor.matmul(kp[:, :], misc_sb[:, _WQK + 65:_WQK + 65 + CQK],
                             xr_sb[:, g * 512:(g + 1) * 512])
            _pcopy(nc, opts, k_sb[:, g * 512:(g + 1) * 512], kp[:, :])

        def em_vprod(g):
            vp = ps.tile([128, 4, 128], f32, tag="misc", bufs=misc_bufs, name="vp")
            for j in range(4):
                ch = 4 * g + j
                nc.tensor.matmul(
                    vp[:, j, 0:66], xbf_sb[:, ch * 128:(ch + 1) * 128], wv_sb[:, :]
                )
            nc.vector.tensor_copy(vT_sb[:, 4 * g:4 * g + 4, :], vp[:, :, 0:65])

        def em_xprod(g):
            xp = ps.tile([128, 4, 128], f32, tag="misc", bufs=misc_bufs, name="xp")
            for j in range(4):
                ch = 4 * g + j
                nc.tensor.transpose(
                    xp[:, j, 0:65],
                    xr_sb[:, ch * 128:(ch + 1) * 128],
                    misc_sb[:, _EYE:_EYE + 65],
                )
            nc.vector.tensor_copy(xT_sb[:, 4 * g:4 * g + 4, :], xp[:, :, 0:65])

        A_ps_holder = []

        def em_amm(i):
            if i == 0:
                A_ps_holder.append(ps.tile(
                    [65, 65], f32, tag="misc", bufs=misc_bufs, name="A_ps"))
            A_ps = A_ps_holder[0]
            nc.tensor.matmul(
                A_ps[:, :], xT_sb[:, i, :], xT_sb[:, i, :],
                start=(i == 0), stop=(i == NCH - 1),
            )

        def em_chain():
            A_ps = A_ps_holder[0]
            m_sb = sb.tile([64, 1], f32, name="m_sb")
            nc.vector.tensor_reduce(m_sb[:, :], A_ps[0:64, 0:64], axis=X, op=Alu.max)
            bm_sb = sb.tile([64, 64], f32, name="bm_sb")
            nc.vector.tensor_scalar(
                bm_sb[:, :], A_ps[0:64, 0:64], m_sb[:, :], None, op0=Alu.subtract
            )
            bt_ps = ps.tile([64, 64], f32, tag="misc", bufs=misc_bufs, name="bt_ps")
            nc.tensor.transpose(bt_ps[:, :], bm_sb[:, :],
                                misc_sb[0:64, _EYE:_EYE + 64])
            mn_sb = sb.tile([64, 1], f32, name="mn_sb")
            nc.vector.tensor_reduce(mn_sb[:, :], bt_ps[:, :], axis=X, op=Alu.min)
            expe_sb = sb.tile([64, 64], f32, name="expe_sb")
            sc_sb = sb.tile([64, 1], f32, name="sc_sb")
            nc.scalar.activation(
                expe_sb[:, :], bt_ps[:, :], AF.Exp,
                scale=-1.0, bias=mn_sb[:, :], accum_out=sc_sb[:, :],
            )
            rc_sb = sb.tile([64, 1], f32, name="rc_sb")
            nc.vector.reciprocal(rc_sb[:, :], sc_sb[:, :])
            att_sb = sb.tile([64, 64], f32, name="att_sb")
            nc.vector.tensor_scalar(
                att_sb[:, :], expe_sb[:, :], rc_sb[:, :],
                misc_sb[0:64, 64:65],
                op0=Alu.mult, op1=Alu.mult,
            )
            att2_sb = sb.tile([64, 64], f32, name="att2_sb")
            nc.vector.tensor_add(att2_sb[:, :], att_sb[:, :],
                                 misc_sb[0:64, 0:64])
            sb._att2 = att2_sb

        def em_cam2(t):
            att2_sb = sb._att2
            cp = ps.tile([65, 512], f32, tag="misc", bufs=misc_bufs, name="cp")
            nc.tensor.matmul(
                cp[0:64, :], att2_sb[:, :], xr_sb[0:64, t * 512:(t + 1) * 512]
            )
            nc.vector.tensor_copy(cam_sb[:, t * 512:(t + 1) * 512], cp[0:64, :])

        # Build the stage-1 work schedule. extras[m] = ops to emit just
        # before global half-group m (m = t*NHG + hg).
        extras = {}

        MLAST = NT * GPT - 1

        def sched(m, fn, *args):
            extras.setdefault(min(m, MLAST), []).append((fn, args))

        if interleave:
            # tile 0: k/v production stays two steps ahead of the pipelined
            # score fills (fill_st runs one group ahead of consumption).
            for g in range(1, KT):
                sched(max(0, (4 * g) // GS - 2), em_kprod, g)
                sched(max(0, (4 * g) // GS - 2), em_vprod, g)
            for t in range(1, NT):
                sched(max(0, (t - 1) * GPT - 2), em_qprod, t)
            # tiles 1-2: transposes; tiles 3-4: gram matmuls; tile 5: chain
            # + cam2 (cam2 must exist before the first deferred tail fires).
            for g in range(KT):
                sched(1 * GPT + (2 * GPT - 2) * g // KT, em_xprod, g)
            for i in range(NCH):
                sched(3 * GPT + (2 * GPT - 2) * i // NCH, em_amm, i)
            sched(5 * GPT, em_chain)
            for t in range(NT):
                sched(5 * GPT + 1 + t, em_cam2, t)
        else:
            for g in range(1, KT):
                sched(0, em_kprod, g)
                sched(0, em_vprod, g)
            for t in range(1, NT):
                sched(0, em_qprod, t)
            for g in range(KT):
                sched(0, em_xprod, g)
            for i in range(NCH):
                sched(0, em_amm, i)
            sched(0, em_chain)
            for t in range(NT):
                sched(0, em_cam2, t)

        # initial productions: enough for tile 0 half-group 0
        em_kprod(0)
        em_qprod(0, first=True)
        em_vprod(0)

        # ---- PAM flash-attention loop ----
        def em_pvc(t, pv):
            pvc = tl.tile([65, 512], f32, tag="pvc", bufs=6, name="pvc")
            nc.vector.tensor_copy(pvc[:, :], pv[:, :])
            return pvc

        def make_tail(t, pvc, split=1):
            def tail():
                rs = tl.tile([1, 512], f32, tag="rs", name="rs")
                nc.vector.reciprocal(rs[:, :], pvc[64:65, :])
                nc.vector.tensor_scalar(
                    rs[:, :], rs[:, :], misc_sb[0:1, 65:66], None, op0=Alu.mult
                )
                w = 512 // split
                for s in range(split):
                    sl = slice(s * w, (s + 1) * w)
                    osl = slice(t * 512 + s * w, t * 512 + (s + 1) * w)
                    bc_sb = tl.tile([64, w], f32, tag=f"bc{split}", bufs=2,
                                    name="bc_sb")
                    nc.gpsimd.partition_broadcast(bc_sb[:, :], rs[0:1, sl])
                    pam_sb = tl.tile([64, w], f32, tag=f"pam{split}", bufs=3,
                                     name="pam_sb")
                    nc.vector.tensor_mul(pam_sb[:, :], pvc[0:64, sl], bc_sb[:, :])
                    out_sb = tl.tile([64, w], f16, tag=f"out{split}", bufs=6,
                                     name="out_sb")
                    nc.vector.tensor_add(
                        out_sb[:, :], pam_sb[:, :], cam_sb[:, osl]
                    )
                    nc.sync.dma_start(y_d[:, osl], out_sb[:, :])
            return tail

        tails = []
        TAILS_OK = 5 * GPT + 2 + NT  # after chain + all cam2 emissions
        M = NT * GPT
        pvs = {}
        sts = {}

        def chunks_of(m):
            t, k = m // GPT, m % GPT
            lo = k * GS
            return t, list(range(lo, min(lo + GS, NCH)))

        def fill_st(m):
            t, chs = chunks_of(m)
            st = ps.tile([128, GS, 512], f32, tag="st", bufs=st_bufs, name="st")
            qs = q_sb[:, t * 512:(t + 1) * 512]
            for j, ch in enumerate(chs):
                nc.tensor.matmul(st[:, j, :], k_sb[:, ch * 128:(ch + 1) * 128], qs)
            sts[m] = st

        pvs[0] = ps.tile([65, 512], f32, tag="pv", bufs=pv_bufs, name="pv")
        fill_st(0)
        for m in range(M):
            t, chs = chunks_of(m)
            k = m % GPT
            pv = pvs[t]
            pt = pps.tile([128, GS, 512], f32r, tag="p", name="pt")
            nc.scalar.activation(
                pt[:, 0:len(chs), :], sts.pop(m)[:, 0:len(chs), :], AF.Exp
            )
            if m + 1 < M:
                if (m + 1) % GPT == 0:
                    pvs[t + 1] = ps.tile([65, 512], f32, tag="pv", bufs=pv_bufs,
                                         name="pv")
                fill_st(m + 1)
            for j, ch in enumerate(chs):
                nc.tensor.matmul(
                    pv[:, :], vT_sb[:, ch, :], pt[:, j, :],
                    start=(m % GPT == 0 and j == 0),
                    stop=(k == GPT - 1 and j == len(chs) - 1),
                )
            for fn, args in extras.pop(m, ()):
                fn(*args)
            # fire deferred tails (they read cam_sb, so not before TAILS_OK)
            while tails and tails[0][0] <= m:
                tails.pop(0)[1]()
            if k == GPT - 1:
                if t == NT - 1:
                    # nothing waits for the last pv slot: the tail reads the
                    # PSUM accumulator directly, skipping the staging copy
                    src_acc = pv
                else:
                    src_acc = em_pvc(t, pv)
                fire_at = max((t + 1) * GPT + 1, TAILS_OK + t)
                tails.append((fire_at, make_tail(t, src_acc,
                                                 split=(2 if t == NT - 1 else 1))))
                del pvs[t]
        for _, fn in tails:
            fn()
        tails.clear()
        assert not extras, f"unscheduled extras: {sorted(extras)}"
    nc.compile()
    return nc


def _get_nc(Ntot, NH, opts=()):
    key = (Ntot, NH, tuple(sorted(dict(opts).items())))
    if key not in _prog_cache:
        _prog_cache[key] = _build(Ntot, NH, opts)
    return _prog_cache[key]


def _core_inputs(xb, w1, b1, w2, b2, w3, b3, gcam, gpam, half, Ntot, NH):
    xroll = np.roll(xb, -half * NH, axis=1)
    s = float(np.abs(xroll).max()) / 32766.0
    xq = np.empty((65, Ntot), np.int16)
    xq[0:64] = np.rint(xroll / s).astype(np.int16)
    xq[64] = np.int16(np.rint(1.0 / s))  # ones row: reconstructs to ~1.0
    misc = np.zeros((65, _MISCW), np.float32)
    misc[0:64, _WQK:_WQK + CQK] = w1.T
    misc[64, _WQK:_WQK + CQK] = b1
    misc[0:64, _WQK + 65:_WQK + 65 + CQK] = w2.T
    misc[64, _WQK + 65:_WQK + 65 + CQK] = b2
    misc[0:64, 0:64] = 2.0 * np.eye(64, dtype=np.float32)
    misc[0:64, 64] = gcam
    misc[0:64, 65] = gpam
    misc[:, _EYE:_EYE + 65] = np.eye(65, dtype=np.float32)
    misc[0:64, _WV:_WV + 64] = w3.T
    misc[64, _WV:_WV + 64] = b3
    misc[64, _WV + 64] = 1.0
    misc[:, _SCL] = s
    return {"xq": xq, "misc": misc}


def kernel(x, w1, b1, w2, b2, w3, b3, gamma_cam, gamma_pam):
    from concourse.bass_utils import run_bass_kernel_spmd

    x = np.asarray(x, dtype=np.float32)
    w1 = np.asarray(w1, dtype=np.float32)
    b1 = np.asarray(b1, dtype=np.float32)
    w2 = np.asarray(w2, dtype=np.float32)
    b2 = np.asarray(b2, dtype=np.float32)
    w3 = np.asarray(w3, dtype=np.float32)
    b3 = np.asarray(b3, dtype=np.float32)
    gcam = float(np.asarray(gamma_cam).reshape(-1)[0])
    gpam = float(np.asarray(gamma_pam).reshape(-1)[0])

    NH = N // 2
    nc = _get_nc(N, NH)
    in_maps = []
    for core in range(NCORES):
        b, half = core // 2, core % 2
        in_maps.append(
            _core_inputs(x[b], w1, b1, w2, b2, w3, b3, gcam, gpam, half, N, NH)
        )
    res = run_bass_kernel_spmd(nc, in_maps, core_ids=list(range(NCORES)))
    y = np.empty((B, C, N), dtype=np.float32)
    for core in range(NCORES):
        b, half = core // 2, core % 2
        y[b, :, half * NH:(half + 1) * NH] = res.results[core]["y"]
    return y

